# revision 35
# baseline (speedup 1.0000x reference)
"""MicroStepDecoder TRN2 kernel (v2).

Math (equivalent to reference via causality/KV-cache):
  gather N=2048 rows -> h0 [N, D]; 5 decode steps of one llama layer,
  step t attends over cached K/V of steps 0..t; output[n, t] = h after step t.

Device strategy: data-parallel over 8 cores, 256 rows/core (2 row-tiles of 128).
All matmuls bf16 (weights bf16 on the wire: halves DMA vs f32r). Transposes via
DMA-engine xbar (dma_start_transpose) instead of PE. W_o resident in SBUF.
Attention on DVE in batched per-(Q-chunk, row-tile) slices overlapped under the
QKV matmuls; per-rt O-projection pipelined against the other rt's attention.
Step 0 skips the Q projection entirely (softmax over 1 element = 1 -> o = v).
RoPE / ln scales / 1/sqrt(hd) folded into weights on host.
"""
import numpy as np
import ml_dtypes

import concourse.bass as bass
import concourse.bacc as bacc
import concourse.tile as tile
import concourse.mybir as mybir
from concourse.masks import make_identity
from concourse.bass_utils import run_bass_kernel_spmd

F32 = mybir.dt.float32
BF16 = mybir.dt.bfloat16
AX = mybir.AxisListType
ALU = mybir.AluOpType
ACTF = mybir.ActivationFunctionType

D = 2048
DFF = 8192
HEADS = 32
KVH = 8
HD = 64
REP = HEADS // KVH
STEPS = 5
NCORES = 8
R = 256            # rows per core
RT = 2             # row tiles per core
KT = D // 128      # 16
EPS = 1e-6
THETA = 1e4
NP_BF16 = ml_dtypes.bfloat16

_CACHE = {}


# ---------------------------------------------------------------- program
def _act_tables_steered(arch):
    """Steer the act-set chooser to the one set holding BOTH Ln and Exp.

    Set ids are untouched (same dict order); Exp/Ln are merely hidden from
    the other sets so the greedy first-match picks natural_log_exp_and_others
    once instead of ping-ponging between exp-only and ln-only sets."""
    import concourse.hw_specs as _hs
    import copy as _copy
    tables = _copy.deepcopy(dict(_hs.get_activation_tables(arch)))
    exp = mybir.ActivationFunctionType.Exp
    ln = mybir.ActivationFunctionType.Ln
    both = {name for name, fs in tables.items() if exp in fs and ln in fs}
    if both:
        for name, fs in tables.items():
            if name not in both:
                fs.discard(exp)
                fs.discard(ln)
    return tables


def _build_program(reps=1):
    bacc.get_activation_tables = _act_tables_steered
    nc = bacc.Bacc("TRN2", target_bir_lowering=False, debug=False)

    h0_d = nc.dram_tensor("h0", [RT, 128, D], F32, kind="ExternalInput")
    qkv_d = nc.dram_tensor("wqkv", [STEPS, 24, 128, 2048], BF16, kind="ExternalInput")
    o_d = nc.dram_tensor("wo", [8, 128, 4096], BF16, kind="ExternalInput")
    g_d = nc.dram_tensor("wg", [32, 128, 4096], BF16, kind="ExternalInput")
    u_d = nc.dram_tensor("wu", [32, 128, 4096], BF16, kind="ExternalInput")
    d_d = nc.dram_tensor("wd", [32, 128, 4096], BF16, kind="ExternalInput")
    out_d = nc.dram_tensor("out", [STEPS, RT, 128, D], F32, kind="ExternalOutput")

    with tile.TileContext(nc) as tc:
        with (
            tc.tile_pool(name="per", bufs=1) as per,
            tc.tile_pool(name="scr", bufs=2) as scr,
            tc.tile_pool(name="qcp", bufs=3) as qcp,
            tc.tile_pool(name="wq_p", bufs=3) as wq_p,
            tc.tile_pool(name="wgu_p", bufs=5) as wgu_p,
            tc.tile_pool(name="wd_p", bufs=3) as wd_p,
            tc.tile_pool(name="wo_p", bufs=4) as wo_p,
            tc.tile_pool(name="sm", bufs=4) as sm,
            tc.tile_pool(name="ps", bufs=4, space=bass.MemorySpace.PSUM) as ps,
            tc.tile_pool(name="ps_gu", bufs=4, space=bass.MemorySpace.PSUM) as ps_gu,
        ):
            eps_t = per.tile([128, 1], F32, tag="eps", name="eps")
            nc.vector.memset(eps_t[:], EPS)
            ident_b = per.tile([128, 128], BF16, tag="identb", name="ident_b")
            make_identity(nc, ident_b[:])

            h = [per.tile([128, D], F32, tag=f"h{rt}", name=f"h{rt}") for rt in range(RT)]
            Kc = [per.tile([128, STEPS, 512], BF16, tag=f"kc{rt}", name=f"kc{rt}")
                  for rt in range(RT)]
            Vc = [per.tile([128, STEPS, 512], BF16, tag=f"vc{rt}", name=f"vc{rt}")
                  for rt in range(RT)]
            oa = [per.tile([128, D], BF16, tag=f"oa{rt}", name=f"oa{rt}") for rt in range(RT)]
            oap = [per.tile([128, 1024], BF16, tag=f"oap{rt}", name=f"oap{rt}")
                   for rt in range(RT)]
            ew = [per.tile([128, STEPS, HEADS], F32, tag=f"e{rt}", name=f"e{rt}")
                  for rt in range(RT)]
            xT = per.tile([128, KT, R], BF16, tag="xT", name="xT")   # doubles as oT
            rstd1 = [per.tile([128, 1], F32, tag=f"r1_{rt}", name=f"r1_{rt}") for rt in range(RT)]
            rstd2 = [per.tile([128, 1], F32, tag=f"r2_{rt}", name=f"r2_{rt}") for rt in range(RT)]
            mT = per.tile([128, 32, R], BF16, tag="mT", name="mT")

            for rt in range(RT):
                nc.sync.dma_start(h[rt][:], h0_d[:][rt])

            def pe_tp_chunk(src_ap, ch, rt):
                tp = ps.tile([128, 512], BF16, tag="mm", name="tp")
                for i in range(4):
                    nc.tensor.transpose(
                        tp[:, i * 128:(i + 1) * 128],
                        src_ap[:, i * 128:(i + 1) * 128], ident_b[:])
                nc.scalar.activation(
                    xT[:, ch * 4:(ch + 1) * 4, rt * 128:(rt + 1) * 128],
                    tp[:].rearrange("p (a c) -> p a c", a=4), func=ACTF.Copy)

            def finish_rstd(parts, dst):
                # combine 4 partial ssq -> rstd = 1/sqrt(ms + eps)
                acc = sm.tile([128, 1], F32, tag="ssq", name="ssq")
                nc.vector.tensor_add(acc[:], parts[0][:], parts[1][:])
                nc.vector.tensor_add(acc[:], acc[:], parts[2][:])
                nc.vector.tensor_add(acc[:], acc[:], parts[3][:])
                sd = sm.tile([128, 1], F32, tag="sd", name="sd")
                nc.scalar.activation(sd[:], acc[:], func=ACTF.Ln,
                                     scale=1.0 / D, bias=eps_t[:])
                nc.scalar.activation(dst[:], sd[:], func=ACTF.Exp, scale=-0.5)

            def norm1_standalone():
                # step-0 path: build xh + rstd1 from fresh h
                for rt in range(RT):
                    parts = []
                    for ch in range(4):
                        sl = slice(ch * 512, (ch + 1) * 512)
                        p = sm.tile([128, 1], F32, tag="sp", name="sp", bufs=10)
                        jk = scr.tile([128, 512], BF16, tag="jk", name="jk", bufs=3)
                        nc.scalar.activation(jk[:], h[rt][:, sl], func=ACTF.Square,
                                             accum_out=p[:])
                        xc = scr.tile([128, 512], BF16, tag="jk", name="xc", bufs=3)
                        nc.scalar.activation(xc[:], h[rt][:, sl], func=ACTF.Copy)
                        pe_tp_chunk(xc[:], ch, rt)
                        parts.append(p)
                    finish_rstd(parts, rstd1[rt])

            def pool_side_pred(ch):
                return ch < 2

            def score_slice(t, ch, rt, qc):
                # kv groups [2ch, 2ch+2), q heads [8ch, 8ch+8)
                kv0 = 2 * ch
                nj = t + 1
                prod = scr.tile([128, STEPS, 2, REP, HD], BF16, tag="prod", name="prod")
                pv = prod[:, :nj]
                for g in range(2):
                    # per kv-group so every AP has <= 3 free dims (DVE ISA limit)
                    qg = qc[:, g * 256:(g + 1) * 256].rearrange(
                        "p (r d) -> p r d", r=REP, d=HD)
                    qg = qg[:, None, :, :].broadcast_to((128, nj, REP, HD))
                    kg = Kc[rt][:, :nj, (kv0 + g) * 64:(kv0 + g + 1) * 64]
                    kg = kg[:, :, None, :].broadcast_to((128, nj, REP, HD))
                    nc.vector.tensor_tensor(pv[:, :, g], qg, kg, op=ALU.mult)
                pvm = pv.rearrange("p j a r d -> p (j a r) d")
                with nc.allow_low_precision(reason="bf16 scores validated 5.5e-3"):
                    nc.vector.tensor_tensor(pvm[:, :, :32], pvm[:, :, :32],
                                            pvm[:, :, 32:], op=ALU.add)
                sc = sm.tile([128, STEPS * 8], BF16, tag="sc", name="sc")
                with nc.allow_low_precision(reason="bf16 scores validated 5.5e-3"):
                    nc.vector.tensor_reduce(
                        sc[:, :nj * 8], pvm[:, :, :32], axis=AX.X, op=ALU.add)
                nc.scalar.activation(
                    ew[rt][:, :nj, ch * 8:(ch + 1) * 8],
                    sc[:, :nj * 8].rearrange("p (j x) -> p j x", x=8),
                    func=ACTF.Exp)
                if not pool_side_pred(ch):
                    # expanded exp over d so the EV mult is fully packed (2x)
                    scb = sc[:, :nj * 8].rearrange("p (x) -> p x", x=nj * 8)[
                        :, :, None].broadcast_to((128, nj * 8, HD))
                    nc.scalar.activation(
                        prod[:, :nj].rearrange("p j a r d -> p (j a r) d"),
                        scb, func=ACTF.Exp)
                # EV accumulate (unnormalized e); Pool owns oap (kv 0-3), DVE owns oa (kv 4-7)
                pool_side = pool_side_pred(ch)
                eng = nc.gpsimd if pool_side else nc.vector
                if pool_side:
                    oa4 = oap[rt][:].rearrange("p (kv r d) -> p kv r d",
                                               kv=4, r=REP)[:, kv0:kv0 + 2]
                else:
                    oa4 = oa[rt][:].rearrange("p (kv r d) -> p kv r d",
                                              kv=KVH, r=REP)[:, kv0:kv0 + 2]
                etag = "evp_p" if pool_side else "evp"
                for j in range(nj):
                    if pool_side:
                        e3 = ew[rt][:, j, ch * 8:(ch + 1) * 8].rearrange(
                            "p (a r) -> p a r", a=2)
                        e4 = e3[:, :, :, None].broadcast_to((128, 2, REP, HD))
                    else:
                        e4 = prod[:, j].rearrange("p a r d -> p a r d")
                    v3 = Vc[rt][:, j, kv0 * 64:(kv0 + 2) * 64].rearrange(
                        "p (a d) -> p a d", a=2)
                    v4 = v3[:, :, None, :].broadcast_to((128, 2, REP, HD))
                    if j == 0:
                        eng.tensor_tensor(oa4, e4, v4, op=ALU.mult)
                    else:
                        evp = scr.tile([128, 512], BF16, tag=etag, name="evp")
                        ev4 = evp[:].rearrange("p (a r d) -> p a r d", a=2, r=REP, d=HD)
                        eng.tensor_tensor(ev4, e4, v4, op=ALU.mult)
                        eng.tensor_tensor(oa4, oa4, ev4, op=ALU.add)

            def attn_final(rt, t):
                # den over j, reciprocal, normalize oa; then transpose into xT
                nj = t + 1
                den = sm.tile([128, HEADS], F32, tag="den", name="den")
                nc.vector.tensor_reduce(
                    den[:], ew[rt][:, :nj, :].rearrange("p j h -> p h j"),
                    axis=AX.X, op=ALU.add)
                rec = sm.tile([128, HEADS], F32, tag="rec", name="rec")
                nc.vector.reciprocal(rec[:], den[:])
                oa4 = oa[rt][:].rearrange("p (kv r d) -> p kv r d", kv=KVH, r=REP)
                op4 = oap[rt][:].rearrange("p (kv r d) -> p kv r d", kv=4, r=REP)
                r4 = rec[:].rearrange("p (kv r) -> p kv r", kv=KVH, r=REP)[
                    :, :, :, None].broadcast_to((128, KVH, REP, HD))
                nc.vector.tensor_tensor(oa4[:, :4], op4, r4[:, :4], op=ALU.mult)
                nc.vector.tensor_tensor(oa4[:, 4:], oa4[:, 4:], r4[:, 4:], op=ALU.mult)
                nc.scalar.dma_start_transpose(
                    xT[:, :, rt * 128:(rt + 1) * 128], oa[rt][:])

            def o_proj():
                # ch-major 1MB wo chunks: chunk (ch, half) covers k in [8h, 8h+8)
                # -> per-ch psums drain while the next ch's matmuls run
                parts = [[], []]
                for ch in range(4):
                    po = [ps.tile([128, 512], F32, tag="mm", name=f"po{_rt}")
                          for _rt in range(RT)]
                    for half in range(2):
                        w = wo_p.tile([128, 4096], BF16, tag="wo", name="wo")
                        nc.sync.dma_start(w[:], o_d[:][ch * 2 + half])
                        for i in range(8):
                            k = half * 8 + i
                            for rt in range(RT):
                                nc.tensor.matmul(
                                    po[rt][:],
                                    xT[:, k, rt * 128:(rt + 1) * 128],
                                    w[:, i * 512:(i + 1) * 512],
                                    start=(k == 0), stop=(k == KT - 1))
                    sl = slice(ch * 512, (ch + 1) * 512)
                    for rt in range(RT):
                        nc.vector.tensor_add(h[rt][:, sl], h[rt][:, sl], po[rt][:])
                        p = sm.tile([128, 1], F32, tag="sp", name="sp", bufs=10)
                        jk = scr.tile([128, 512], BF16, tag="jk", name="jk", bufs=3)
                        nc.scalar.activation(jk[:], h[rt][:, sl], func=ACTF.Square,
                                             accum_out=p[:])
                        parts[rt].append(p)
                for rt in range(RT):
                    finish_rstd(parts[rt], rstd2[rt])
                for ch in range(4):
                    for rt in range(RT):
                        sl = slice(ch * 512, (ch + 1) * 512)
                        xc = scr.tile([128, 512], BF16, tag="jk", name="xc", bufs=3)
                        nc.scalar.activation(xc[:], h[rt][:, sl], func=ACTF.Copy,
                                             scale=rstd2[rt][:])
                        pe_tp_chunk(xc[:], ch, rt)

            def qkv_chunk_mms(t, ch):
                pq = [ps.tile([128, 512], F32, tag="mm", name=f"pq{_rt}")
                      for _rt in range(RT)]
                for kg in range(4):
                    w = wq_p.tile([128, 2048], BF16, tag="wq", name="wq")
                    nc.sync.dma_start(w[:], qkv_d[:][t, ch * 4 + kg])
                    for i in range(4):
                        k = kg * 4 + i
                        for rt in range(RT):
                            nc.tensor.matmul(
                                pq[rt][:], xT[:, k, rt * 128:(rt + 1) * 128],
                                w[:, i * 512:(i + 1) * 512],
                                start=(k == 0), stop=(k == KT - 1))
                return pq

            def qkv_phase(t):
                # ch 4 = K, ch 5 = V, ch 0..3 = Q (skipped at t=0)
                for ch in ([4, 5] if t == 0 else [4, 5, 0, 1, 2, 3]):
                    pq = qkv_chunk_mms(t, ch)
                    if ch >= 4:
                        for rt in range(RT):
                            dst = Kc[rt][:, t, :] if ch == 4 else Vc[rt][:, t, :]
                            nc.scalar.activation(dst, pq[rt][:], func=ACTF.Copy,
                                                 scale=rstd1[rt][:])
                    else:
                        for rt in range(RT):
                            qc = qcp.tile([128, 512], BF16, tag="qc", name="qc")
                            nc.scalar.activation(qc[:], pq[rt][:], func=ACTF.Copy,
                                                 scale=rstd1[rt][:])
                            score_slice(t, ch, rt, qc)
                if t == 0:
                    for rt in range(RT):
                        oa4 = oa[rt][:].rearrange("p (kv r d) -> p kv r d", kv=KVH, r=REP)
                        v4 = Vc[rt][:, 0, :].rearrange("p (kv d) -> p kv d", kv=KVH)[
                            :, :, None, :].broadcast_to((128, KVH, REP, HD))
                        nc.vector.tensor_copy(oa4, v4)
                        nc.scalar.dma_start_transpose(
                            xT[:, :, rt * 128:(rt + 1) * 128], oa[rt][:])
                else:
                    for rt in range(RT):
                        attn_final(rt, t)
                o_proj()

            def mlp_phase():
                s1parts = [[], []]
                for half in range(2):
                    for pr in range(16 * half, 16 * (half + 1)):
                        wg_t = wgu_p.tile([128, 4096], BF16, tag="wgu", name="wgt")
                        nc.gpsimd.dma_start(wg_t[:], g_d[:][pr])
                        wu_t = wgu_p.tile([128, 4096], BF16, tag="wgu", name="wut")
                        nc.gpsimd.dma_start(wu_t[:], u_d[:][pr])
                        for mgi in range(2):
                            mloc = (pr * 2 + mgi) - 32 * half
                            pg = ps_gu.tile([128, R], F32, tag="gu", name="pg")
                            for k in range(KT):
                                c = (mgi * KT + k) * 128
                                nc.tensor.matmul(
                                    pg[:], wg_t[:, c:c + 128], xT[:, k, :],
                                    start=(k == 0), stop=(k == KT - 1))
                            pu = ps_gu.tile([128, R], F32, tag="gu", name="pu")
                            for k in range(KT):
                                c = (mgi * KT + k) * 128
                                nc.tensor.matmul(
                                    pu[:], wu_t[:, c:c + 128], xT[:, k, :],
                                    start=(k == 0), stop=(k == KT - 1))
                            sg = sm.tile([128, R], BF16, tag="sg", name="sg")
                            nc.scalar.activation(sg[:], pg[:], func=ACTF.Silu)
                            nc.vector.tensor_tensor(
                                mT[:, mloc, :], sg[:], pu[:], op=ALU.mult)
                    for ch in range(4):
                        pd_ = [ps.tile([128, 512], F32, tag="mm", name=f"pd{_rt}")
                               for _rt in range(RT)]
                        for kfg in range(4 * half, 4 * (half + 1)):
                            w = wd_p.tile([128, 4096], BF16, tag="wd", name="wdt")
                            dge = nc.sync if kfg % 2 == 0 else nc.scalar
                            dge.dma_start(w[:], d_d[:][ch * 8 + kfg])
                            for i in range(8):
                                kf = kfg * 8 + i
                                kfl = kf - 32 * half
                                for rt in range(RT):
                                    nc.tensor.matmul(
                                        pd_[rt][:],
                                        mT[:, kfl, rt * 128:(rt + 1) * 128],
                                        w[:, i * 512:(i + 1) * 512],
                                        start=(kf == 32 * half),
                                        stop=(kf == 32 * half + 31))
                        for rt in range(RT):
                            sl = slice(ch * 512, (ch + 1) * 512)
                            nc.vector.tensor_add(h[rt][:, sl], h[rt][:, sl],
                                                 pd_[rt][:])
                            if half == 1:
                                p = sm.tile([128, 1], F32, tag="sp", name="sp", bufs=10)
                                jk = scr.tile([128, 512], BF16, tag="jk",
                                              name="jk", bufs=3)
                                nc.scalar.activation(jk[:], h[rt][:, sl],
                                                     func=ACTF.Square,
                                                     accum_out=p[:])
                                xc = scr.tile([128, 512], BF16, tag="jk",
                                              name="xc", bufs=3)
                                nc.scalar.activation(xc[:], h[rt][:, sl],
                                                     func=ACTF.Copy)
                                pe_tp_chunk(xc[:], ch, rt)
                                s1parts[rt].append(p)
                return s1parts

            def mlp_finish(s1parts):
                for rt in range(RT):
                    finish_rstd(s1parts[rt], rstd1[rt])

            first = True
            for t in [t for _ in range(reps) for t in range(STEPS)]:
                if first:
                    norm1_standalone()
                    first = False
                qkv_phase(t)
                s1parts = mlp_phase()
                mlp_finish(s1parts)
                for rt in range(RT):
                    nc.gpsimd.dma_start(out_d[:][t, rt], h[rt][:])

    nc.compile()
    return nc


# ---------------------------------------------------------------- host prep
def _rope_cs(t):
    inv = 1.0 / (THETA ** (np.arange(0, HD, 2, dtype=np.float64) / HD))
    emb = np.concatenate([t * inv, t * inv])
    return np.cos(emb), np.sin(emb)


def _rope_cols(w, t, nheads):
    w3 = w.reshape(D, nheads, HD)
    cos, sin = _rope_cs(t)
    wrot = np.concatenate([-w3[:, :, HD // 2:], w3[:, :, :HD // 2]], axis=2)
    return (w3 * cos[None, None, :] + wrot * sin[None, None, :]).reshape(D, nheads * HD)


def _pack_rhs(w, n_ch, n_kg):
    # w [K, n_ch*512]; chunks (ch, kg): [128, 4*512]; kg covers 4 k-tiles
    kt = w.shape[0] // 128
    A = w.reshape(n_kg, kt // n_kg, 128, n_ch, 512)
    return np.ascontiguousarray(A.transpose(3, 0, 2, 1, 4)).reshape(
        n_ch * n_kg, 128, (kt // n_kg) * 512)


def _pack_wo_kmajor(w):
    # w [D, D] -> 8 chunks [128, (k 8, 512)]; chunk (ch, half) covers ch cols,
    # k-tiles [8*half, 8*half+8)
    A = w.reshape(2, 8, 128, 4, 512)
    return np.ascontiguousarray(A.transpose(3, 0, 2, 1, 4)).reshape(8, 128, 4096)


def _pack_lhs_gu(w):
    # w [D, DFF] -> [32 pairs][128, (mgi 2, k 16, 128)]
    B = w.reshape(KT, 128, 32, 2, 128)
    return np.ascontiguousarray(B.transpose(2, 1, 3, 0, 4)).reshape(32, 128, 4096)


def _pack_rhs_dn(w):
    # w [DFF, D] -> chunks (ch 4, kfg 8): [128, (i 8, 512)]
    C = w.reshape(8, 8, 128, 4, 512)
    return np.ascontiguousarray(C.transpose(3, 0, 2, 1, 4)).reshape(32, 128, 4096)


def _gather_indices(comp_seq_lens, inst_lens):
    seqs = np.asarray(comp_seq_lens)
    insts = np.asarray(inst_lens)
    idx, off = [], 0
    for s, i in zip(seqs, insts):
        s, i = int(s), int(i)
        idx.append(np.arange(off + i - 1, off + s - 1))
        off += s
    return np.concatenate(idx)


def _prep_inputs(hidden_states, comp_seq_lens, inst_lens, w_q, w_k, w_v, w_o,
                 ln1_w, ln2_w, w_gate, w_up, w_down):
    idx = _gather_indices(comp_seq_lens, inst_lens)
    h0 = np.asarray(hidden_states, np.float32)[0, idx]          # [N, D]
    N = h0.shape[0]
    assert N == NCORES * R, f"expected {NCORES*R} rows, got {N}"

    ln1 = np.asarray(ln1_w, np.float64)
    ln2 = np.asarray(ln2_w, np.float64)
    wq_e = np.asarray(w_q, np.float64) * ln1[:, None] * (HD ** -0.5)
    wk_e = np.asarray(w_k, np.float64) * ln1[:, None]
    wv_e = np.asarray(w_v, np.float64) * ln1[:, None]
    wg_e = np.asarray(w_gate, np.float64) * ln2[:, None]
    wu_e = np.asarray(w_up, np.float64) * ln2[:, None]

    qkv_pack = np.empty((STEPS, 24, 128, 2048), NP_BF16)
    for t in range(STEPS):
        wq_t = _rope_cols(wq_e, t, HEADS)
        wk_t = _rope_cols(wk_e, t, KVH)
        qkv = np.concatenate([wq_t, wk_t, wv_e], axis=1).astype(np.float32)
        qkv_pack[t] = _pack_rhs(qkv, 6, 4).astype(NP_BF16)

    weights = {
        "wqkv": qkv_pack,
        "wo": _pack_wo_kmajor(np.asarray(w_o, np.float32)).astype(NP_BF16),
        "wg": _pack_lhs_gu(wg_e.astype(np.float32)).astype(NP_BF16),
        "wu": _pack_lhs_gu(wu_e.astype(np.float32)).astype(NP_BF16),
        "wd": _pack_rhs_dn(np.asarray(w_down, np.float32)).astype(NP_BF16),
    }
    h0_cores = h0.reshape(NCORES, RT, 128, D)
    return weights, h0_cores


def kernel(**inputs):
    weights, h0_cores = _prep_inputs(**inputs)

    if "nc" not in _CACHE:
        _CACHE["nc"] = _build_program()
    nc = _CACHE["nc"]

    in_maps = [dict(weights, h0=np.ascontiguousarray(h0_cores[c]))
               for c in range(NCORES)]
    res = run_bass_kernel_spmd(nc, in_maps, core_ids=list(range(NCORES)))
    _CACHE["last_results"] = res

    outs = []
    for c in range(NCORES):
        o = res.results[c]["out"]                  # [5, RT, 128, D]
        outs.append(o.reshape(STEPS, R, D).transpose(1, 0, 2))
    return np.concatenate(outs, axis=0)            # [N, 5, D]


# revision 39
# speedup vs baseline: 1845.4713x; 1845.4713x over previous
"""MicroStepDecoder TRN2 kernel (v2).

Math (equivalent to reference via causality/KV-cache):
  gather N=2048 rows -> h0 [N, D]; 5 decode steps of one llama layer,
  step t attends over cached K/V of steps 0..t; output[n, t] = h after step t.

Device strategy: data-parallel over 8 cores, 256 rows/core (2 row-tiles of 128).
All matmuls bf16 (weights bf16 on the wire: halves DMA vs f32r). Transposes via
DMA-engine xbar (dma_start_transpose) instead of PE. W_o resident in SBUF.
Attention on DVE in batched per-(Q-chunk, row-tile) slices overlapped under the
QKV matmuls; per-rt O-projection pipelined against the other rt's attention.
Step 0 skips the Q projection entirely (softmax over 1 element = 1 -> o = v).
RoPE / ln scales / 1/sqrt(hd) folded into weights on host.
"""
import numpy as np
import ml_dtypes

import concourse.bass as bass
import concourse.bacc as bacc
import concourse.tile as tile
import concourse.mybir as mybir
from concourse.masks import make_identity
from concourse.bass_utils import run_bass_kernel_spmd

F32 = mybir.dt.float32
BF16 = mybir.dt.bfloat16
AX = mybir.AxisListType
ALU = mybir.AluOpType
ACTF = mybir.ActivationFunctionType

D = 2048
DFF = 8192
HEADS = 32
KVH = 8
HD = 64
REP = HEADS // KVH
STEPS = 5
NCORES = 8
R = 256            # rows per core
RT = 2             # row tiles per core
KT = D // 128      # 16
EPS = 1e-6
THETA = 1e4
NP_BF16 = ml_dtypes.bfloat16

_CACHE = {}


# ---------------------------------------------------------------- program
def _act_tables_steered(arch):
    """Steer the act-set chooser to the one set holding BOTH Ln and Exp.

    Set ids are untouched (same dict order); Exp/Ln are merely hidden from
    the other sets so the greedy first-match picks natural_log_exp_and_others
    once instead of ping-ponging between exp-only and ln-only sets."""
    import concourse.hw_specs as _hs
    import copy as _copy
    tables = _copy.deepcopy(dict(_hs.get_activation_tables(arch)))
    exp = mybir.ActivationFunctionType.Exp
    ln = mybir.ActivationFunctionType.Ln
    both = {name for name, fs in tables.items() if exp in fs and ln in fs}
    if both:
        for name, fs in tables.items():
            if name not in both:
                fs.discard(exp)
                fs.discard(ln)
    return tables


def _build_program(reps=1):
    bacc.get_activation_tables = _act_tables_steered
    nc = bacc.Bacc("TRN2", target_bir_lowering=False, debug=False)

    h0_d = nc.dram_tensor("h0", [RT, 128, D], F32, kind="ExternalInput")
    qkv_d = nc.dram_tensor("wqkv", [STEPS, 24, 128, 2048], BF16, kind="ExternalInput")
    o_d = nc.dram_tensor("wo", [8, 128, 4096], BF16, kind="ExternalInput")
    g_d = nc.dram_tensor("wg", [32, 128, 4096], BF16, kind="ExternalInput")
    u_d = nc.dram_tensor("wu", [32, 128, 4096], BF16, kind="ExternalInput")
    d_d = nc.dram_tensor("wd", [32, 128, 4096], BF16, kind="ExternalInput")
    out_d = nc.dram_tensor("out", [STEPS, RT, 128, D], F32, kind="ExternalOutput")

    with tile.TileContext(nc) as tc:
        with (
            tc.tile_pool(name="per", bufs=1) as per,
            tc.tile_pool(name="scr", bufs=2) as scr,
            tc.tile_pool(name="qcp", bufs=3) as qcp,
            tc.tile_pool(name="wq_p", bufs=3) as wq_p,
            tc.tile_pool(name="wgu_p", bufs=5) as wgu_p,
            tc.tile_pool(name="wd_p", bufs=3) as wd_p,
            tc.tile_pool(name="wo_p", bufs=3) as wo_p,
            tc.tile_pool(name="sm", bufs=4) as sm,
            tc.tile_pool(name="ps", bufs=4, space=bass.MemorySpace.PSUM) as ps,
            tc.tile_pool(name="ps_gu", bufs=4, space=bass.MemorySpace.PSUM) as ps_gu,
        ):
            eps_t = per.tile([128, 1], F32, tag="eps", name="eps")
            nc.vector.memset(eps_t[:], EPS)
            ident_b = per.tile([128, 128], BF16, tag="identb", name="ident_b")
            make_identity(nc, ident_b[:])

            h = [per.tile([128, D], F32, tag=f"h{rt}", name=f"h{rt}") for rt in range(RT)]
            Kc = [per.tile([128, STEPS, 512], BF16, tag=f"kc{rt}", name=f"kc{rt}")
                  for rt in range(RT)]
            Vc = [per.tile([128, STEPS, 512], BF16, tag=f"vc{rt}", name=f"vc{rt}")
                  for rt in range(RT)]
            oa = [per.tile([128, D], BF16, tag=f"oa{rt}", name=f"oa{rt}") for rt in range(RT)]
            oap = [per.tile([128, 1024], BF16, tag=f"oap{rt}", name=f"oap{rt}")
                   for rt in range(RT)]
            ew = [per.tile([128, STEPS, HEADS], F32, tag=f"e{rt}", name=f"e{rt}")
                  for rt in range(RT)]
            xT = per.tile([128, KT, R], BF16, tag="xT", name="xT")   # doubles as oT
            rstd1 = [per.tile([128, 1], F32, tag=f"r1_{rt}", name=f"r1_{rt}") for rt in range(RT)]
            rstd2 = [per.tile([128, 1], F32, tag=f"r2_{rt}", name=f"r2_{rt}") for rt in range(RT)]
            mT = per.tile([128, 32, R], BF16, tag="mT", name="mT")

            for rt in range(RT):
                nc.sync.dma_start(h[rt][:], h0_d[:][rt])

            def pe_tp_chunk(src_ap, ch, rt):
                tp = ps.tile([128, 512], BF16, tag="mm", name="tp")
                for i in range(4):
                    nc.tensor.transpose(
                        tp[:, i * 128:(i + 1) * 128],
                        src_ap[:, i * 128:(i + 1) * 128], ident_b[:])
                nc.scalar.activation(
                    xT[:, ch * 4:(ch + 1) * 4, rt * 128:(rt + 1) * 128],
                    tp[:].rearrange("p (a c) -> p a c", a=4), func=ACTF.Copy)

            def finish_rstd(parts, dst):
                # combine 4 partial ssq -> rstd = 1/sqrt(ms + eps)
                acc = sm.tile([128, 1], F32, tag="ssq", name="ssq")
                nc.vector.tensor_add(acc[:], parts[0][:], parts[1][:])
                nc.vector.tensor_add(acc[:], acc[:], parts[2][:])
                nc.vector.tensor_add(acc[:], acc[:], parts[3][:])
                sd = sm.tile([128, 1], F32, tag="sd", name="sd")
                nc.scalar.activation(sd[:], acc[:], func=ACTF.Ln,
                                     scale=1.0 / D, bias=eps_t[:])
                nc.scalar.activation(dst[:], sd[:], func=ACTF.Exp, scale=-0.5)

            def norm1_standalone():
                # step-0 path: build xh + rstd1 from fresh h
                for rt in range(RT):
                    parts = []
                    for ch in range(4):
                        sl = slice(ch * 512, (ch + 1) * 512)
                        p = sm.tile([128, 1], F32, tag="sp", name="sp", bufs=10)
                        jk = scr.tile([128, 512], BF16, tag="jk", name="jk", bufs=3)
                        nc.scalar.activation(jk[:], h[rt][:, sl], func=ACTF.Square,
                                             accum_out=p[:])
                        xc = scr.tile([128, 512], BF16, tag="jk", name="xc", bufs=3)
                        nc.scalar.activation(xc[:], h[rt][:, sl], func=ACTF.Copy)
                        pe_tp_chunk(xc[:], ch, rt)
                        parts.append(p)
                    finish_rstd(parts, rstd1[rt])

            def pool_side_pred(ch):
                return ch < 2

            def score_slice(t, ch, rt, qc, emit_ev=True):
                # kv groups [2ch, 2ch+2), q heads [8ch, 8ch+8)
                kv0 = 2 * ch
                nj = t + 1
                prod = scr.tile([128, STEPS, 2, REP, HD], BF16, tag="prod", name="prod")
                pv = prod[:, :nj]
                for g in range(2):
                    # per kv-group so every AP has <= 3 free dims (DVE ISA limit)
                    qg = qc[:, g * 256:(g + 1) * 256].rearrange(
                        "p (r d) -> p r d", r=REP, d=HD)
                    qg = qg[:, None, :, :].broadcast_to((128, nj, REP, HD))
                    kg = Kc[rt][:, :nj, (kv0 + g) * 64:(kv0 + g + 1) * 64]
                    kg = kg[:, :, None, :].broadcast_to((128, nj, REP, HD))
                    nc.vector.tensor_tensor(pv[:, :, g], qg, kg, op=ALU.mult)
                pvm = pv.rearrange("p j a r d -> p (j a r) d")
                with nc.allow_low_precision(reason="bf16 scores validated 5.5e-3"):
                    nc.vector.tensor_tensor(pvm[:, :, :32], pvm[:, :, :32],
                                            pvm[:, :, 32:], op=ALU.add)
                sc = sm.tile([128, STEPS * 8], BF16, tag="sc", name="sc")
                with nc.allow_low_precision(reason="bf16 scores validated 5.5e-3"):
                    nc.vector.tensor_reduce(
                        sc[:, :nj * 8], pvm[:, :, :32], axis=AX.X, op=ALU.add)
                nc.scalar.activation(
                    ew[rt][:, :nj, ch * 8:(ch + 1) * 8],
                    sc[:, :nj * 8].rearrange("p (j x) -> p j x", x=8),
                    func=ACTF.Exp)
                if not pool_side_pred(ch):
                    # expanded exp over d so the EV mult is fully packed (2x)
                    scb = sc[:, :nj * 8].rearrange("p (x) -> p x", x=nj * 8)[
                        :, :, None].broadcast_to((128, nj * 8, HD))
                    nc.scalar.activation(
                        prod[:, :nj].rearrange("p j a r d -> p (j a r) d"),
                        scb, func=ACTF.Exp)
                if emit_ev:
                    ev_slice(t, ch, rt, prod)

            def ev_slice(t, ch, rt, prod=None):
                kv0 = 2 * ch
                nj = t + 1
                # EV accumulate (unnormalized e); Pool owns oap (kv 0-3), DVE owns oa (kv 4-7)
                pool_side = pool_side_pred(ch)
                eng = nc.gpsimd if pool_side else nc.vector
                if pool_side:
                    oa4 = oap[rt][:].rearrange("p (kv r d) -> p kv r d",
                                               kv=4, r=REP)[:, kv0:kv0 + 2]
                else:
                    oa4 = oa[rt][:].rearrange("p (kv r d) -> p kv r d",
                                              kv=KVH, r=REP)[:, kv0:kv0 + 2]
                etag = "evp_p" if pool_side else "evp"
                for j in range(nj):
                    if pool_side:
                        e3 = ew[rt][:, j, ch * 8:(ch + 1) * 8].rearrange(
                            "p (a r) -> p a r", a=2)
                        e4 = e3[:, :, :, None].broadcast_to((128, 2, REP, HD))
                    else:
                        e4 = prod[:, j].rearrange("p a r d -> p a r d")
                    v3 = Vc[rt][:, j, kv0 * 64:(kv0 + 2) * 64].rearrange(
                        "p (a d) -> p a d", a=2)
                    v4 = v3[:, :, None, :].broadcast_to((128, 2, REP, HD))
                    if j == 0:
                        eng.tensor_tensor(oa4, e4, v4, op=ALU.mult)
                    else:
                        evp = scr.tile([128, 512], BF16, tag=etag, name="evp")
                        ev4 = evp[:].rearrange("p (a r d) -> p a r d", a=2, r=REP, d=HD)
                        eng.tensor_tensor(ev4, e4, v4, op=ALU.mult)
                        eng.tensor_tensor(oa4, oa4, ev4, op=ALU.add)

            def attn_final(rt, t):
                # den over j, reciprocal, normalize oa; then transpose into xT
                nj = t + 1
                den = sm.tile([128, HEADS], F32, tag="den", name="den")
                nc.vector.tensor_reduce(
                    den[:], ew[rt][:, :nj, :].rearrange("p j h -> p h j"),
                    axis=AX.X, op=ALU.add)
                rec = sm.tile([128, HEADS], F32, tag="rec", name="rec")
                nc.vector.reciprocal(rec[:], den[:])
                oa4 = oa[rt][:].rearrange("p (kv r d) -> p kv r d", kv=KVH, r=REP)
                op4 = oap[rt][:].rearrange("p (kv r d) -> p kv r d", kv=4, r=REP)
                r4 = rec[:].rearrange("p (kv r) -> p kv r", kv=KVH, r=REP)[
                    :, :, :, None].broadcast_to((128, KVH, REP, HD))
                nc.vector.tensor_tensor(oa4[:, :4], op4, r4[:, :4], op=ALU.mult)
                nc.vector.tensor_tensor(oa4[:, 4:], oa4[:, 4:], r4[:, 4:], op=ALU.mult)
                for c in range(4):
                    pe_tp_chunk(oa[rt][:, c * 512:(c + 1) * 512], c, rt)

            def o_proj():
                # ch-major 1MB wo chunks: chunk (ch, half) covers k in [8h, 8h+8)
                # -> per-ch psums drain while the next ch's matmuls run
                parts = [[], []]
                for ch in range(4):
                    po = [ps.tile([128, 512], F32, tag="mm", name=f"po{_rt}")
                          for _rt in range(RT)]
                    for half in range(2):
                        w = wo_p.tile([128, 4096], BF16, tag="wo", name="wo")
                        nc.sync.dma_start(w[:], o_d[:][ch * 2 + half])
                        for i in range(8):
                            k = half * 8 + i
                            for rt in range(RT):
                                nc.tensor.matmul(
                                    po[rt][:],
                                    xT[:, k, rt * 128:(rt + 1) * 128],
                                    w[:, i * 512:(i + 1) * 512],
                                    start=(k == 0), stop=(k == KT - 1))
                    sl = slice(ch * 512, (ch + 1) * 512)
                    for rt in range(RT):
                        nc.vector.tensor_add(h[rt][:, sl], h[rt][:, sl], po[rt][:])
                        p = sm.tile([128, 1], F32, tag="sp", name="sp", bufs=10)
                        jk = scr.tile([128, 512], BF16, tag="jk", name="jk", bufs=3)
                        nc.scalar.activation(jk[:], h[rt][:, sl], func=ACTF.Square,
                                             accum_out=p[:])
                        parts[rt].append(p)
                for rt in range(RT):
                    finish_rstd(parts[rt], rstd2[rt])
                for ch in range(4):
                    for rt in range(RT):
                        sl = slice(ch * 512, (ch + 1) * 512)
                        xc = scr.tile([128, 512], BF16, tag="jk", name="xc", bufs=3)
                        nc.scalar.activation(xc[:], h[rt][:, sl], func=ACTF.Copy,
                                             scale=rstd2[rt][:])
                        pe_tp_chunk(xc[:], ch, rt)

            def qkv_chunk_mms(t, ch):
                pq = [ps.tile([128, 512], F32, tag="mm", name=f"pq{_rt}")
                      for _rt in range(RT)]
                for kg in range(4):
                    w = wq_p.tile([128, 2048], BF16, tag="wq", name="wq")
                    nc.sync.dma_start(w[:], qkv_d[:][t, ch * 4 + kg])
                    for i in range(4):
                        k = kg * 4 + i
                        for rt in range(RT):
                            nc.tensor.matmul(
                                pq[rt][:], xT[:, k, rt * 128:(rt + 1) * 128],
                                w[:, i * 512:(i + 1) * 512],
                                start=(k == 0), stop=(k == KT - 1))
                return pq

            def qkv_phase(t):
                # ch 4 = K, ch 5 = V, ch 0..3 = Q (skipped at t=0)
                for ch in ([4, 5] if t == 0 else [4, 0, 5, 1, 2, 3]):
                    pq = qkv_chunk_mms(t, ch)
                    if ch >= 4:
                        for rt in range(RT):
                            dst = Kc[rt][:, t, :] if ch == 4 else Vc[rt][:, t, :]
                            nc.scalar.activation(dst, pq[rt][:], func=ACTF.Copy,
                                                 scale=rstd1[rt][:])
                        if ch == 5 and t > 0:
                            for rt in range(RT):
                                ev_slice(t, 0, rt)   # ch0 is Pool-side: EV after V drain
                    else:
                        for rt in range(RT):
                            qc = qcp.tile([128, 512], BF16, tag="qc", name="qc")
                            nc.scalar.activation(qc[:], pq[rt][:], func=ACTF.Copy,
                                                 scale=rstd1[rt][:])
                            score_slice(t, ch, rt, qc, emit_ev=(ch != 0))
                if t == 0:
                    for rt in range(RT):
                        oa4 = oa[rt][:].rearrange("p (kv r d) -> p kv r d", kv=KVH, r=REP)
                        v4 = Vc[rt][:, 0, :].rearrange("p (kv d) -> p kv d", kv=KVH)[
                            :, :, None, :].broadcast_to((128, KVH, REP, HD))
                        nc.vector.tensor_copy(oa4, v4)
                        for c in range(4):
                            pe_tp_chunk(oa[rt][:, c * 512:(c + 1) * 512], c, rt)
                else:
                    for rt in range(RT):
                        attn_final(rt, t)
                o_proj()

            def mlp_phase():
                s1parts = [[], []]
                for half in range(2):
                    for pr in range(16 * half, 16 * (half + 1)):
                        wg_t = wgu_p.tile([128, 4096], BF16, tag="wgu", name="wgt")
                        nc.gpsimd.dma_start(wg_t[:], g_d[:][pr])
                        wu_t = wgu_p.tile([128, 4096], BF16, tag="wgu", name="wut")
                        nc.gpsimd.dma_start(wu_t[:], u_d[:][pr])
                        for mgi in range(2):
                            mloc = (pr * 2 + mgi) - 32 * half
                            pg = ps_gu.tile([128, R], F32, tag="gu", name="pg")
                            for k in range(KT):
                                c = (mgi * KT + k) * 128
                                nc.tensor.matmul(
                                    pg[:], wg_t[:, c:c + 128], xT[:, k, :],
                                    start=(k == 0), stop=(k == KT - 1))
                            pu = ps_gu.tile([128, R], F32, tag="gu", name="pu")
                            for k in range(KT):
                                c = (mgi * KT + k) * 128
                                nc.tensor.matmul(
                                    pu[:], wu_t[:, c:c + 128], xT[:, k, :],
                                    start=(k == 0), stop=(k == KT - 1))
                            sg = sm.tile([128, R], BF16, tag="sg", name="sg")
                            nc.scalar.activation(sg[:], pg[:], func=ACTF.Silu)
                            nc.vector.tensor_tensor(
                                mT[:, mloc, :], sg[:], pu[:], op=ALU.mult)
                    for ch in range(4):
                        pd_ = [ps.tile([128, 512], F32, tag="mm", name=f"pd{_rt}")
                               for _rt in range(RT)]
                        for kfg in range(4 * half, 4 * (half + 1)):
                            w = wd_p.tile([128, 4096], BF16, tag="wd", name="wdt")
                            dge = nc.sync if kfg % 2 == 0 else nc.scalar
                            dge.dma_start(w[:], d_d[:][ch * 8 + kfg])
                            for i in range(8):
                                kf = kfg * 8 + i
                                kfl = kf - 32 * half
                                for rt in range(RT):
                                    nc.tensor.matmul(
                                        pd_[rt][:],
                                        mT[:, kfl, rt * 128:(rt + 1) * 128],
                                        w[:, i * 512:(i + 1) * 512],
                                        start=(kf == 32 * half),
                                        stop=(kf == 32 * half + 31))
                        for rt in range(RT):
                            sl = slice(ch * 512, (ch + 1) * 512)
                            nc.vector.tensor_add(h[rt][:, sl], h[rt][:, sl],
                                                 pd_[rt][:])
                            if half == 1:
                                p = sm.tile([128, 1], F32, tag="sp", name="sp", bufs=10)
                                jk = scr.tile([128, 512], BF16, tag="jk",
                                              name="jk", bufs=3)
                                nc.scalar.activation(jk[:], h[rt][:, sl],
                                                     func=ACTF.Square,
                                                     accum_out=p[:])
                                xc = scr.tile([128, 512], BF16, tag="jk",
                                              name="xc", bufs=3)
                                nc.scalar.activation(xc[:], h[rt][:, sl],
                                                     func=ACTF.Copy)
                                pe_tp_chunk(xc[:], ch, rt)
                                s1parts[rt].append(p)
                return s1parts

            def mlp_finish(s1parts):
                for rt in range(RT):
                    finish_rstd(s1parts[rt], rstd1[rt])

            first = True
            for t in [t for _ in range(reps) for t in range(STEPS)]:
                if first:
                    norm1_standalone()
                    first = False
                qkv_phase(t)
                s1parts = mlp_phase()
                mlp_finish(s1parts)
                for rt in range(RT):
                    nc.gpsimd.dma_start(out_d[:][t, rt], h[rt][:])

    nc.compile()
    return nc


# ---------------------------------------------------------------- host prep
def _rope_cs(t):
    inv = 1.0 / (THETA ** (np.arange(0, HD, 2, dtype=np.float64) / HD))
    emb = np.concatenate([t * inv, t * inv])
    return np.cos(emb), np.sin(emb)


def _rope_cols(w, t, nheads):
    w3 = w.reshape(D, nheads, HD)
    cos, sin = _rope_cs(t)
    wrot = np.concatenate([-w3[:, :, HD // 2:], w3[:, :, :HD // 2]], axis=2)
    return (w3 * cos[None, None, :] + wrot * sin[None, None, :]).reshape(D, nheads * HD)


def _pack_rhs(w, n_ch, n_kg):
    # w [K, n_ch*512]; chunks (ch, kg): [128, 4*512]; kg covers 4 k-tiles
    kt = w.shape[0] // 128
    A = w.reshape(n_kg, kt // n_kg, 128, n_ch, 512)
    return np.ascontiguousarray(A.transpose(3, 0, 2, 1, 4)).reshape(
        n_ch * n_kg, 128, (kt // n_kg) * 512)


def _pack_wo_kmajor(w):
    # w [D, D] -> 8 chunks [128, (k 8, 512)]; chunk (ch, half) covers ch cols,
    # k-tiles [8*half, 8*half+8)
    A = w.reshape(2, 8, 128, 4, 512)
    return np.ascontiguousarray(A.transpose(3, 0, 2, 1, 4)).reshape(8, 128, 4096)


def _pack_lhs_gu(w):
    # w [D, DFF] -> [32 pairs][128, (mgi 2, k 16, 128)]
    B = w.reshape(KT, 128, 32, 2, 128)
    return np.ascontiguousarray(B.transpose(2, 1, 3, 0, 4)).reshape(32, 128, 4096)


def _pack_rhs_dn(w):
    # w [DFF, D] -> chunks (ch 4, kfg 8): [128, (i 8, 512)]
    C = w.reshape(8, 8, 128, 4, 512)
    return np.ascontiguousarray(C.transpose(3, 0, 2, 1, 4)).reshape(32, 128, 4096)


def _gather_indices(comp_seq_lens, inst_lens):
    seqs = np.asarray(comp_seq_lens)
    insts = np.asarray(inst_lens)
    idx, off = [], 0
    for s, i in zip(seqs, insts):
        s, i = int(s), int(i)
        idx.append(np.arange(off + i - 1, off + s - 1))
        off += s
    return np.concatenate(idx)


def _prep_inputs(hidden_states, comp_seq_lens, inst_lens, w_q, w_k, w_v, w_o,
                 ln1_w, ln2_w, w_gate, w_up, w_down):
    idx = _gather_indices(comp_seq_lens, inst_lens)
    h0 = np.asarray(hidden_states, np.float32)[0, idx]          # [N, D]
    N = h0.shape[0]
    assert N == NCORES * R, f"expected {NCORES*R} rows, got {N}"

    ln1 = np.asarray(ln1_w, np.float64)
    ln2 = np.asarray(ln2_w, np.float64)
    wq_e = np.asarray(w_q, np.float64) * ln1[:, None] * (HD ** -0.5)
    wk_e = np.asarray(w_k, np.float64) * ln1[:, None]
    wv_e = np.asarray(w_v, np.float64) * ln1[:, None]
    wg_e = np.asarray(w_gate, np.float64) * ln2[:, None]
    wu_e = np.asarray(w_up, np.float64) * ln2[:, None]

    qkv_pack = np.empty((STEPS, 24, 128, 2048), NP_BF16)
    for t in range(STEPS):
        wq_t = _rope_cols(wq_e, t, HEADS)
        wk_t = _rope_cols(wk_e, t, KVH)
        qkv = np.concatenate([wq_t, wk_t, wv_e], axis=1).astype(np.float32)
        qkv_pack[t] = _pack_rhs(qkv, 6, 4).astype(NP_BF16)

    weights = {
        "wqkv": qkv_pack,
        "wo": _pack_wo_kmajor(np.asarray(w_o, np.float32)).astype(NP_BF16),
        "wg": _pack_lhs_gu(wg_e.astype(np.float32)).astype(NP_BF16),
        "wu": _pack_lhs_gu(wu_e.astype(np.float32)).astype(NP_BF16),
        "wd": _pack_rhs_dn(np.asarray(w_down, np.float32)).astype(NP_BF16),
    }
    h0_cores = h0.reshape(NCORES, RT, 128, D)
    return weights, h0_cores


def kernel(**inputs):
    weights, h0_cores = _prep_inputs(**inputs)

    if "nc" not in _CACHE:
        _CACHE["nc"] = _build_program()
    nc = _CACHE["nc"]

    in_maps = [dict(weights, h0=np.ascontiguousarray(h0_cores[c]))
               for c in range(NCORES)]
    res = run_bass_kernel_spmd(nc, in_maps, core_ids=list(range(NCORES)))
    _CACHE["last_results"] = res

    outs = []
    for c in range(NCORES):
        o = res.results[c]["out"]                  # [5, RT, 128, D]
        outs.append(o.reshape(STEPS, R, D).transpose(1, 0, 2))
    return np.concatenate(outs, axis=0)            # [N, 5, D]
